# revision 2
# baseline (speedup 1.0000x reference)
"""Bahdanau-attention kernel for 8 Trainium2 NeuronCores (SPMD, batch-sharded).

Algorithm: scores[t,s] = sum_h v_h * tanh(D[h,t] + E[h,s]) via a structured
sine expansion  tanh(x) ~= b1 sin(w1 x) + b2 sin(2 w1 x) + b3 sin(3 w1 x)
+ b4 sin(4 w1 x) + b5 sin(w2 x)  (Gaussian-weighted LSQ fit on the actual
arg distribution, sigma~=1.41), factored through the angle-addition formula
into 20 PSUM-accumulating fp16 matmuls over sin/cos features of
uD = W2^T dec^T and uE = W1^T enc^T.

Only the two base pairs hit the ACT engine's Sin LUT:
  w2 pair: |w2 u| + pi/2 < 3.55 -> direct Sin, no range reduction.
  w1 pair: magic-constant range reduction on DVE in f32 (a = v - round(v),
           b = |a| via sign-bit mask), then Sin(2*pi*a), Sin(-2*pi*b + pi/2).
Harmonics 2w1/3w1/4w1 are derived on DVE from s1/c1 with exact double/triple
angle identities in fp16 (tensor ops get 2x/4x DVE modes for 16-bit):
  q=s1^2; s2'=s1*c1 (=sin2/2, fold 2 into b2); c2=1-2q;
  s3=s1*(3-4q); c3=c1*(1-4q); s4'=s2'*c2 (=sin4/4, fold 4); c4=1-8*s2'^2.
v*b_k folds into the decoder-side features as [128,256] fp16 tensor_scalars
on the otherwise-idle GpSimd (Pool) engine. The encoder padding mask enters
PSUM as a (-60000|-2) seed via K=1 rank-1 fp16 matmuls (the -2 shifts exp
into fp16 range); softmax runs without max-shift (fp16 exp, f32 accum_out
row sums); the decoder mask folds into the 1/sum scale; output is stored
fp16 and upcast on host.

Inputs ride two parallel HWDGE rings: [W1|encT] + mask row on the Sync ring,
[W2|decT|vb:dm-bits] on the Activation ring (the f32 vb/dm rides as raw
bits, bitcast on device, with explicit add_dep edges on its readers).
Outputs are split across the two rings the same way. Transposes/casts are
host-side layout prep.
"""
import os
import sys

import numpy as np

if "/opt/trn_rl_repo" not in sys.path:
    sys.path.insert(0, "/opt/trn_rl_repo")

S, T, B, H = 512, 256, 8, 128

# Gaussian-weighted LSQ fit of tanh on sigma=1.414 (see module docstring).
W1F = 0.8200000000000001
W2F = 0.27000000000000013
BK = np.array(
    [0.39725026, 0.14210005, 0.02599986, 0.0149857, 1.18657541],
    dtype=np.float64,
)
# effective fold coefficients: sin2 = 2*s2', sin4 = 4*s4'
BEFF = np.array(
    [BK[0], 2.0 * BK[1], BK[2], 4.0 * BK[3], BK[4]], dtype=np.float64
)
TWO_PI = float(2.0 * np.pi)
HALF_PI = float(0.5 * np.pi)
SCAL1 = float(W1F / (2.0 * np.pi))
M32 = float(1.5 * 2**23)
MASK_NEG = -60000.0
MASK_POS = -2.0  # uniform exp shift keeps fp16 exp in range

_CACHE = {}
LAST_EXEC_NS = None


def _try_install_trace_hook():
    """Best-effort NTFF profile hook for axon (used only when tracing)."""
    try:
        import contextlib
        import ctypes
        import types

        if "antenv.axon_hooks" in sys.modules:
            return
        lib = ctypes.CDLL("/opt/axon/libaxon_pjrt.so")
        if not hasattr(lib, "axon_start_nrt_profile"):
            return
        lib.axon_start_nrt_profile.argtypes = [
            ctypes.POINTER(ctypes.c_int64),
            ctypes.c_size_t,
        ]
        lib.axon_start_nrt_profile.restype = ctypes.c_int64
        lib.axon_stop_nrt_profile.argtypes = [ctypes.c_char_p]
        lib.axon_stop_nrt_profile.restype = ctypes.c_int64

        @contextlib.contextmanager
        def _hook(output_dir, device_ids):
            import jax

            jax.devices()
            if device_ids:
                ids = (ctypes.c_int64 * len(device_ids))(*device_ids)
                rc = lib.axon_start_nrt_profile(ids, len(device_ids))
            else:
                rc = lib.axon_start_nrt_profile(None, 0)
            if rc != 0:
                raise RuntimeError(f"axon_start_nrt_profile rc={rc}")
            try:
                yield
            finally:
                n = lib.axon_stop_nrt_profile(str(output_dir).encode())
                if n < 0:
                    raise RuntimeError(f"axon_stop_nrt_profile rc={n}")

        mod = types.ModuleType("antenv.axon_hooks")
        _h = _hook

        def set_axon_ntff_profile_hook(h):
            pass

        def get_axon_ntff_profile_hook():
            return _h

        mod.set_axon_ntff_profile_hook = set_axon_ntff_profile_hook
        mod.get_axon_ntff_profile_hook = get_axon_ntff_profile_hook
        sys.modules["antenv.axon_hooks"] = mod
        import antenv

        antenv.axon_hooks = mod
    except Exception:
        pass


def _build():
    if "nc" in _CACHE:
        return _CACHE["nc"]
    import concourse.bacc as bacc
    import concourse.tile as tile
    from concourse.tile import add_dep_helper
    import concourse.mybir as mybir

    F32 = mybir.dt.float32
    U16 = mybir.dt.uint16
    FP16 = mybir.dt.float16
    AF = mybir.ActivationFunctionType
    AL = mybir.AluOpType

    nc = bacc.Bacc("TRN2", target_bir_lowering=False, debug=False, num_devices=8)

    NF = 5  # frequencies
    P3C = NF + 2  # f32 cols: vb per freq + dm halves
    PK1C = (H + T) + 2 * P3C
    pk2_d = nc.dram_tensor("pack2", [H, H + S], FP16, kind="ExternalInput")
    pk1_d = nc.dram_tensor("pack1", [H, PK1C], FP16, kind="ExternalInput")
    em_d = nc.dram_tensor("encmask", [1, S], FP16, kind="ExternalInput")
    out_d = nc.dram_tensor("out", [T, S], FP16, kind="ExternalOutput")

    W = S + T  # 768: feature tiles are [e(512) | d(256)]

    with tile.TileContext(nc) as tc:
        with (
            tc.tile_pool(name="cst", bufs=1) as cst,
            tc.tile_pool(name="wrk", bufs=1) as wrk,
            tc.tile_pool(name="ps", bufs=1, space="PSUM") as psp,
        ):
            # ---- inputs on two parallel HWDGE rings: [W1|encT]+mask on the
            # Sync ring, [W2|decT|vb:dm-bits] on the Activation ring ----
            with nc.named_scope("dma_in"):
                pk2_sb = cst.tile([H, H + S], FP16)
                nc.sync.dma_start(pk2_sb[:], pk2_d[:])
                em_sb = cst.tile([1, S], FP16)
                nc.sync.dma_start(em_sb[:], em_d[:])
                pk1_sb = cst.tile([H, PK1C], FP16)
                pk_dma = nc.scalar.dma_start(pk1_sb[:], pk1_d[:])

            p1 = pk1_sb[:, 0:H + T]
            p2 = pk2_sb[:]
            p3 = pk1_sb[:, H + T:PK1C].bitcast(F32)

            ones_sb = cst.tile([1, H], FP16)
            nc.gpsimd.memset(ones_sb[:], 1.0)
            hp_sb = cst.tile([128, 1], F32)
            nc.gpsimd.memset(hp_sb[:], HALF_PI)

            # ---- u matmuls into two PSUM tensors (parallel PSUM->SBUF
            # copies: DVE for e, ACT for d) ----
            uE_ps = psp.tile([128, S], F32, tag="upsE")
            uD_ps = psp.tile([128, T], F32, tag="upsD")
            with nc.named_scope("u_mm"):
                nc.tensor.matmul(
                    uE_ps[:], p2[:, 0:H], p2[:, H:], start=True, stop=True)
                nc.tensor.matmul(
                    uD_ps[:], p1[:, 0:H], p1[:, H:], start=True, stop=True)
            u_sb = wrk.tile([128, W], FP16, name="u_sb")
            with nc.named_scope("u_copy"):
                nc.vector.tensor_scalar_mul(u_sb[:, 0:S], uE_ps[:], 1.0)
                nc.scalar.copy(u_sb[:, S:], uD_ps[:])

            # score PSUM seeded with the (-2|-60000) encoder mask
            sc = []
            for tb in range(2):
                sc_tile = psp.tile([128, S], F32, tag=f"sc{tb}")
                sc.append(sc_tile)
                with nc.named_scope(f"mask_{tb}"):
                    nc.tensor.matmul(
                        sc_tile[:], ones_sb[:], em_sb[:],
                        start=True, stop=False, skip_group_check=True,
                    )

            # ---- ACT stream: w2 pair direct from u ----
            with nc.named_scope("sin_w2"):
                sw2 = wrk.tile([128, W], FP16, name="sw2")
                nc.scalar.activation(sw2[:], u_sb[:], AF.Sin, scale=W2F)
                cw2 = wrk.tile([128, W], FP16, name="cw2")
                nc.scalar.activation(
                    cw2[:], u_sb[:], AF.Sin, bias=hp_sb[:], scale=W2F)

            # ---- w1 range reduction on DVE (f32 magic-constant round) ----
            with nc.named_scope("red_w1"):
                vv = wrk.tile([128, W], F32, name="vv")
                nc.vector.tensor_scalar_mul(vv[:], u_sb[:], SCAL1)
                ii = wrk.tile([128, W], F32, name="ii")
                nc.vector.tensor_scalar(
                    ii[:], vv[:], M32, M32, AL.add, AL.subtract)
                aa = wrk.tile([128, W], FP16, name="aa")
                nc.vector.tensor_tensor(aa[:], vv[:], ii[:], AL.subtract)
                bb = wrk.tile([128, W], FP16, name="bb")
                bb_i = nc.vector.tensor_scalar(
                    bb[:].bitcast(U16), aa[:].bitcast(U16), 0x7FFF,
                    None, AL.bitwise_and)

            with nc.named_scope("sin_w1"):
                s1 = wrk.tile([128, W], FP16, name="s1")
                nc.scalar.activation(s1[:], aa[:], AF.Sin, scale=TWO_PI)
                c1 = wrk.tile([128, W], FP16, name="c1")
                c1_i = nc.scalar.activation(
                    c1[:], bb[:], AF.Sin, bias=hp_sb[:], scale=-TWO_PI)
            add_dep_helper(c1_i.ins, bb_i.ins, reason="c1 reads sign-masked bb")

            # ---- DVE harmonic ladder (exact identities, fp16) ----
            with nc.named_scope("harm"):
                q = wrk.tile([128, W], FP16, name="q")
                nc.vector.tensor_tensor(q[:], s1[:], s1[:], AL.mult)
                s2p = wrk.tile([128, W], FP16, name="s2p")
                nc.vector.tensor_tensor(s2p[:], s1[:], c1[:], AL.mult)
                c2 = wrk.tile([128, W], FP16, name="c2")
                nc.vector.tensor_scalar(
                    c2[:], q[:], -2.0, 1.0, AL.mult, AL.add)
                t3 = wrk.tile([128, W], FP16, name="t3")
                nc.vector.tensor_scalar(
                    t3[:], q[:], -4.0, 3.0, AL.mult, AL.add)
                r3 = wrk.tile([128, W], FP16, name="r3")
                nc.vector.tensor_scalar(
                    r3[:], q[:], -4.0, 1.0, AL.mult, AL.add)
                s3 = wrk.tile([128, W], FP16, name="s3")
                nc.vector.tensor_tensor(s3[:], s1[:], t3[:], AL.mult)
                c3 = wrk.tile([128, W], FP16, name="c3")
                nc.vector.tensor_tensor(c3[:], c1[:], r3[:], AL.mult)
                s4p = wrk.tile([128, W], FP16, name="s4p")
                nc.vector.tensor_tensor(s4p[:], s2p[:], c2[:], AL.mult)
                qq = wrk.tile([128, W], FP16, name="qq")
                nc.vector.tensor_tensor(qq[:], s2p[:], s2p[:], AL.mult)
                c4 = wrk.tile([128, W], FP16, name="c4")
                nc.vector.tensor_scalar(
                    c4[:], qq[:], -8.0, 1.0, AL.mult, AL.add)

            # ---- folds on GpSimd (Pool): d-side features * (v*b_k) ----
            def folds(k, fS, fC):
                with nc.named_scope(f"vfold_{k}"):
                    fSdv = wrk.tile([128, T], FP16, name=f"fSdv{k}")
                    i1 = nc.gpsimd.tensor_scalar_mul(
                        fSdv[:], fS[:, S:], p3[:, k:k + 1])
                    fCdv = wrk.tile([128, T], FP16, name=f"fCdv{k}")
                    i2 = nc.gpsimd.tensor_scalar_mul(
                        fCdv[:], fC[:, S:], p3[:, k:k + 1])
                add_dep_helper(i1.ins, pk_dma.ins, reason="p3 bitcast read after DMA")
                add_dep_helper(i2.ins, pk_dma.ins, reason="p3 bitcast read after DMA")
                return fSdv, fCdv

            def scores(k, fSdv, fCdv, fS, fC, last=False):
                with nc.named_scope(f"scores_{k}"):
                    for tb in range(2):
                        dsl = slice(tb * 128, (tb + 1) * 128)
                        nc.tensor.matmul(
                            sc[tb][:], fSdv[:, dsl], fC[:, 0:S],
                            start=False, stop=False, skip_group_check=True,
                        )
                        nc.tensor.matmul(
                            sc[tb][:], fCdv[:, dsl], fS[:, 0:S],
                            start=False, stop=last, skip_group_check=True,
                        )

            # freq order: w2 (features earliest), w1, 2w1, 3w1, 4w1
            fSw2, fCw2 = folds(4, sw2, cw2)
            scores("w2", fSw2, fCw2, sw2, cw2)
            fS1, fC1 = folds(0, s1, c1)
            scores("w1", fS1, fC1, s1, c1)
            fS2, fC2 = folds(1, s2p, c2)
            scores("h2", fS2, fC2, s2p, c2)
            fS3, fC3 = folds(2, s3, c3)
            scores("h3", fS3, fC3, s3, c3)
            fS4, fC4 = folds(3, s4p, c4)
            scores("h4", fS4, fC4, s4p, c4, last=True)

            # ---- softmax + decoder-mask scale + store; outputs split
            # across the two HWDGE rings ----
            ex, rs = {}, {}
            for tb in range(2):
                with nc.named_scope(f"exp_{tb}"):
                    ex[tb] = wrk.tile([128, S], FP16, name=f"ex{tb}")
                    rs[tb] = wrk.tile([128, 1], F32, name=f"rs{tb}")
                    nc.scalar.activation(
                        ex[tb][:], sc[tb][:], AF.Exp, accum_out=rs[tb][:])
            for tb in range(2):
                with nc.named_scope(f"scale_{tb}"):
                    ri = wrk.tile([128, 1], F32, name=f"ri{tb}")
                    nc.vector.reciprocal(ri[:], rs[tb][:])
                    fac = wrk.tile([128, 1], F32, name=f"fac{tb}")
                    fac_i = nc.vector.tensor_tensor(
                        fac[:], ri[:], p3[:, NF + tb:NF + tb + 1],
                        mybir.AluOpType.mult)
                    add_dep_helper(fac_i.ins, pk_dma.ins, reason="p3 bitcast read")
                    ot = wrk.tile([128, S], FP16, name=f"ot{tb}")
                    nc.vector.tensor_scalar_mul(ot[:], ex[tb][:], fac[:])
                    eng = nc.sync if tb == 0 else nc.scalar
                    eng.dma_start(out_d[tb * 128:(tb + 1) * 128, :], ot[:])

    nc.compile()
    _CACHE["nc"] = nc
    return nc


def kernel(encoder_output, decoder_output, W1, W2, v, enc_lens, dec_lens):
    global LAST_EXEC_NS
    from concourse.bass_utils import run_bass_kernel_spmd

    FP = np.float16
    enc = np.asarray(encoder_output, dtype=np.float32)
    dec = np.asarray(decoder_output, dtype=np.float32)
    W1 = np.asarray(W1, dtype=np.float32)
    W2 = np.asarray(W2, dtype=np.float32)
    v = np.asarray(v, dtype=np.float32)
    enc_lens = np.asarray(enc_lens)
    dec_lens = np.asarray(dec_lens)

    vb = (v[:, None].astype(np.float64) * BEFF[None, :]).astype(np.float32)  # (H,5)

    in_maps = []
    for b in range(B):
        p1 = np.concatenate([W2, dec[:, b, :].T], axis=1).astype(FP)
        p2 = np.ascontiguousarray(
            np.concatenate([W1, enc[:, b, :].T], axis=1).astype(FP))
        dm = (np.arange(T) < int(dec_lens[b])).astype(np.float32)
        p3 = np.ascontiguousarray(
            np.concatenate([vb, dm.reshape(H, 2, order="F")], axis=1),
            dtype=np.float32)
        p3_bits = p3.view(np.uint16).view(FP)  # raw f32 bits as fp16 pairs
        pack1 = np.ascontiguousarray(np.concatenate([p1, p3_bits], axis=1))
        em = np.where(
            np.arange(S)[None, :] < int(enc_lens[b]), MASK_POS, MASK_NEG
        ).astype(FP)
        in_maps.append({"pack1": pack1, "pack2": p2, "encmask": em})

    trace = os.environ.get("KERNEL_TRACE", "0") == "1"
    if trace:
        _try_install_trace_hook()
    nc = _build()
    ncores = int(os.environ.get("KERNEL_CORES", str(B)))
    res = run_bass_kernel_spmd(nc, in_maps[:ncores], core_ids=list(range(ncores)), trace=trace)
    if trace:
        LAST_EXEC_NS = res.exec_time_ns
        _CACHE["last_res"] = res

    out = np.zeros((T, B, S), dtype=np.float32)
    for b in range(ncores):
        out[:, b, :] = np.asarray(res.results[b]["out"], dtype=np.float32)
    return out


# revision 4
# speedup vs baseline: 2.2480x; 2.2480x over previous
"""Bahdanau-attention kernel for 8 Trainium2 NeuronCores (SPMD, batch-sharded).

Algorithm: scores[t,s] = sum_h v_h * tanh(D[h,t] + E[h,s]) via a structured
sine expansion  tanh(x) ~= b1 sin(w1 x) + b2 sin(2 w1 x) + b3 sin(3 w1 x)
+ b4 sin(4 w1 x) + b5 sin(w2 x)  (Gaussian-weighted LSQ fit on the actual
arg distribution, sigma~=1.41), factored through the angle-addition formula
into 20 PSUM-accumulating bf16 matmuls over sin/cos features of
uD = W2^T dec^T and uE = W1^T enc^T.

Only the two base pairs hit the ACT engine's Sin LUT (w1*|u| exceeds the
~+-3.55 LUT domain for only ~3e-5 of elements, which clamp benignly; the
cos args never leave the domain):
  s1 = Sin(w1 u),  c1 = Sin(-w1 |u| + pi/2)   (|u| via one sign-mask op)
  sw2 = Sin(w2 u), cw2 = Sin(w2 u + pi/2)
Harmonics 2w1/3w1/4w1 are derived on DVE from s1/c1 with exact double/triple
angle identities in bf16, SPLIT into encoder-side [128,512] features and
decoder-side [128,256] features with the v*b_k fold fused into the d-side
tensor_scalars (per-partition f32 scalar columns shipped in the pack):
  e: q=s1^2; s2'=s1c1; c2=1-2q; s3=s1(3-4q); c3=c1(1-4q); s4'=s2'c2;
     c4=1-8s2'^2   (sin2=2s2', sin4=4s4' -> the 2/4 fold into b2/b4)
  d: the same values pre-multiplied by vb_k via fused (q*a+b)-style ops.
The encoder padding mask enters PSUM as a -1e30 seed via K=1 rank-1 bf16
matmuls during the input-DMA shadow; softmax runs without max-shift (bf16
exp, f32 accum_out row sums); the decoder mask folds into the 1/sum scale;
output is stored bf16 and upcast on host.

Inputs ride two parallel HWDGE rings: [W1|encT] + mask row on the Sync ring,
[W2|decT|scalars:dm-bits] on the Activation ring (the f32 scalar columns
ride as raw bits, bitcast on device, with explicit add_dep edges on their
readers). Outputs are split across the two rings the same way.
"""
import os
import sys

import numpy as np

if "/opt/trn_rl_repo" not in sys.path:
    sys.path.insert(0, "/opt/trn_rl_repo")

S, T, B, H = 512, 256, 8, 128

# Gaussian-weighted LSQ fit of tanh on sigma=1.414 (see module docstring).
W1F = 0.8200000000000001
W2F = 0.27000000000000013
BK = np.array(
    [0.39725026, 0.14210005, 0.02599986, 0.0149857, 1.18657541],
    dtype=np.float64,
)
# effective fold coefficients: sin2 = 2*s2', sin4 = 4*s4'
BEFF = np.array(
    [BK[0], 2.0 * BK[1], BK[2], 4.0 * BK[3], BK[4]], dtype=np.float64
)
TWO_PI = float(2.0 * np.pi)
HALF_PI = float(0.5 * np.pi)
NEG_BIG = -1.0e30

_CACHE = {}
LAST_EXEC_NS = None


def _try_install_trace_hook():
    """Best-effort NTFF profile hook for axon (used only when tracing)."""
    try:
        import contextlib
        import ctypes
        import types

        if "antenv.axon_hooks" in sys.modules:
            return
        lib = ctypes.CDLL("/opt/axon/libaxon_pjrt.so")
        if not hasattr(lib, "axon_start_nrt_profile"):
            return
        lib.axon_start_nrt_profile.argtypes = [
            ctypes.POINTER(ctypes.c_int64),
            ctypes.c_size_t,
        ]
        lib.axon_start_nrt_profile.restype = ctypes.c_int64
        lib.axon_stop_nrt_profile.argtypes = [ctypes.c_char_p]
        lib.axon_stop_nrt_profile.restype = ctypes.c_int64

        @contextlib.contextmanager
        def _hook(output_dir, device_ids):
            import jax

            jax.devices()
            if device_ids:
                ids = (ctypes.c_int64 * len(device_ids))(*device_ids)
                rc = lib.axon_start_nrt_profile(ids, len(device_ids))
            else:
                rc = lib.axon_start_nrt_profile(None, 0)
            if rc != 0:
                raise RuntimeError(f"axon_start_nrt_profile rc={rc}")
            try:
                yield
            finally:
                n = lib.axon_stop_nrt_profile(str(output_dir).encode())
                if n < 0:
                    raise RuntimeError(f"axon_stop_nrt_profile rc={n}")

        mod = types.ModuleType("antenv.axon_hooks")
        _h = _hook

        def set_axon_ntff_profile_hook(h):
            pass

        def get_axon_ntff_profile_hook():
            return _h

        mod.set_axon_ntff_profile_hook = set_axon_ntff_profile_hook
        mod.get_axon_ntff_profile_hook = get_axon_ntff_profile_hook
        sys.modules["antenv.axon_hooks"] = mod
        import antenv

        antenv.axon_hooks = mod
    except Exception:
        pass


# f32 per-partition scalar columns shipped in pack1 (order matters):
# vb0, vb1, m2vb1, vb2m4, vb2p3, vb2, m2vb3, vb3, m8vb3, vb4, dm0, dm1
NSCAL = 12


def _build():
    if "nc" in _CACHE:
        return _CACHE["nc"]
    import concourse.bacc as bacc
    import concourse.tile as tile
    from concourse.tile import add_dep_helper
    import concourse.mybir as mybir

    F32 = mybir.dt.float32
    U32 = mybir.dt.uint32
    BF16 = mybir.dt.bfloat16
    AF = mybir.ActivationFunctionType
    AL = mybir.AluOpType

    nc = bacc.Bacc("TRN2", target_bir_lowering=False, debug=False, num_devices=8)

    PK1C = (H + T) + 2 * NSCAL
    pk2_d = nc.dram_tensor("pack2", [H, H + S], BF16, kind="ExternalInput")
    pk1_d = nc.dram_tensor("pack1", [H, PK1C], BF16, kind="ExternalInput")
    em_d = nc.dram_tensor("encmask", [1, S], BF16, kind="ExternalInput")
    out_d = nc.dram_tensor("out", [T, S], BF16, kind="ExternalOutput")

    W = S + T  # 768

    with tile.TileContext(nc) as tc:
        with (
            tc.tile_pool(name="cst", bufs=1) as cst,
            tc.tile_pool(name="wrk", bufs=1) as wrk,
            tc.tile_pool(name="ps", bufs=1, space="PSUM") as psp,
        ):
            # ---- inputs on two parallel HWDGE rings ----
            with nc.named_scope("dma_in"):
                pk2_sb = cst.tile([H, H + S], BF16)
                nc.sync.dma_start(pk2_sb[:], pk2_d[:])
                em_sb = cst.tile([1, S], BF16)
                nc.sync.dma_start(em_sb[:], em_d[:])
                pk1_sb = cst.tile([H, PK1C], BF16)
                pk_dma = nc.scalar.dma_start(pk1_sb[:], pk1_d[:])

            p1 = pk1_sb[:, 0:H + T]
            p2 = pk2_sb[:]
            p3 = pk1_sb[:, H + T:PK1C].bitcast(F32)  # [128, NSCAL] f32

            def scal(idx):
                return p3[:, idx:idx + 1]

            ones_sb = cst.tile([1, H], BF16)
            nc.gpsimd.memset(ones_sb[:], 1.0)
            hp_sb = cst.tile([128, 1], F32)
            nc.gpsimd.memset(hp_sb[:], HALF_PI)

            # ---- u matmuls into two PSUM tensors ----
            uE_ps = psp.tile([128, S], F32, tag="upsE")
            uD_ps = psp.tile([128, T], F32, tag="upsD")
            with nc.named_scope("u_mm"):
                nc.tensor.matmul(
                    uE_ps[:], p2[:, 0:H], p2[:, H:], start=True, stop=True)
                nc.tensor.matmul(
                    uD_ps[:], p1[:, 0:H], p1[:, H:], start=True, stop=True)
            u_sb = wrk.tile([128, W], F32, name="u_sb")
            with nc.named_scope("u_copy"):
                nc.vector.tensor_scalar_mul(u_sb[:, 0:S], uE_ps[:], 1.0)
                nc.scalar.copy(u_sb[:, S:], uD_ps[:])

            # score PSUM seeded with -1e30 encoder mask
            sc = []
            for tb in range(2):
                sc_tile = psp.tile([128, S], F32, tag=f"sc{tb}")
                sc.append(sc_tile)
                with nc.named_scope(f"mask_{tb}"):
                    nc.tensor.matmul(
                        sc_tile[:], ones_sb[:], em_sb[:],
                        start=True, stop=False, skip_group_check=True,
                    )

            # |u| for the c1 arg (sign-bit mask on f32)
            absu = wrk.tile([128, W], F32, name="absu")
            absu_i = nc.vector.tensor_scalar(
                absu[:].bitcast(U32), u_sb[:].bitcast(U32), 0x7FFFFFFF, None,
                AL.bitwise_and)

            # ---- ACT stream: base pairs (w1 first so DVE starts early) ----
            with nc.named_scope("sin_w1"):
                s1 = wrk.tile([128, W], BF16, name="s1")
                nc.scalar.activation(s1[:], u_sb[:], AF.Sin, scale=W1F)
                c1 = wrk.tile([128, W], BF16, name="c1")
                c1_i = nc.scalar.activation(
                    c1[:], absu[:], AF.Sin, bias=hp_sb[:], scale=-W1F)
            add_dep_helper(c1_i.ins, absu_i.ins, reason="c1 reads sign-masked absu")
            with nc.named_scope("sin_w2"):
                sw2 = wrk.tile([128, W], BF16, name="sw2")
                nc.scalar.activation(sw2[:], u_sb[:], AF.Sin, scale=W2F)
                cw2 = wrk.tile([128, W], BF16, name="cw2")
                nc.scalar.activation(
                    cw2[:], u_sb[:], AF.Sin, bias=hp_sb[:], scale=W2F)

            dE = slice(0, S)      # encoder cols of a [128,768] feature
            dD = slice(S, W)      # decoder cols

            def tt(name, a, b, cols, dt=BF16):
                t = wrk.tile([128, cols], dt, name=name)
                i = nc.vector.tensor_tensor(t[:], a, b, AL.mult)
                return t, i

            def ts2(name, a, s1_, s2_, cols, dt=BF16):
                """t = a*s1_ + s2_ (per-partition f32 scalars or floats)."""
                t = wrk.tile([128, cols], dt, name=name)
                i = nc.vector.tensor_scalar(t[:], a, s1_, s2_, AL.mult, AL.add)
                return t, i

            def tsm(name, a, s, cols, dt=BF16):
                t = wrk.tile([128, cols], dt, name=name)
                i = nc.vector.tensor_scalar_mul(t[:], a, s)
                return t, i

            def scores(k, lhS, lhC, cos_e, sin_e, last=False):
                """sc += lhS^T x cos_e + lhC^T x sin_e (lh* are folded d-side)."""
                with nc.named_scope(f"scores_{k}"):
                    for tb in range(2):
                        dsl = slice(tb * 128, (tb + 1) * 128)
                        nc.tensor.matmul(
                            sc[tb][:], lhS[:, dsl], cos_e,
                            start=False, stop=False, skip_group_check=True,
                        )
                        nc.tensor.matmul(
                            sc[tb][:], lhC[:, dsl], sin_e,
                            start=False, stop=last, skip_group_check=True,
                        )

            # ---- DVE harmonic ladder, e/d split with fused folds ----
            # freq w1: direct folds
            with nc.named_scope("fold_w1"):
                fS1dv, i1 = tsm("fS1dv", s1[:, dD], scal(0), T)
                fC1dv, i2 = tsm("fC1dv", c1[:, dD], scal(0), T)
            add_dep_helper(i1.ins, pk_dma.ins, reason="p3 bitcast read")
            add_dep_helper(i2.ins, pk_dma.ins, reason="p3 bitcast read")
            scores("w1", fS1dv, fC1dv, c1[:, dE], s1[:, dE])

            with nc.named_scope("harm_shared"):
                qe, _ = tt("qe", s1[:, dE], s1[:, dE], S)
                qd, _ = tt("qd", s1[:, dD], s1[:, dD], T)
                s2pe, _ = tt("s2pe", s1[:, dE], c1[:, dE], S)
                s2pd, _ = tt("s2pd", s1[:, dD], c1[:, dD], T)

            # freq 2w1: e = (s2', c2), d folded
            with nc.named_scope("harm_h2"):
                c2e, _ = ts2("c2e", qe[:], -2.0, 1.0, S)
                fS2dv, i3 = tsm("fS2dv", s2pd[:], scal(1), T)
                fC2dv, i4 = ts2("fC2dv", qd[:], scal(2), scal(1), T)
            add_dep_helper(i3.ins, pk_dma.ins, reason="p3 bitcast read")
            add_dep_helper(i4.ins, pk_dma.ins, reason="p3 bitcast read")
            scores("h2", fS2dv, fC2dv, c2e[:], s2pe[:])

            # freq 3w1: s3 = s1(3-4q), c3 = c1(1-4q)
            with nc.named_scope("harm_h3"):
                t3e, _ = ts2("t3e", qe[:], -4.0, 3.0, S)
                s3e, _ = tt("s3e", s1[:, dE], t3e[:], S)
                r3e, _ = ts2("r3e", qe[:], -4.0, 1.0, S)
                c3e, _ = tt("c3e", c1[:, dE], r3e[:], S)
                t3dv, i5 = ts2("t3dv", qd[:], scal(3), scal(4), T)
                fS3dv, _ = tt("fS3dv", t3dv[:], s1[:, dD], T)
                r3dv, i6 = ts2("r3dv", qd[:], scal(3), scal(5), T)
                fC3dv, _ = tt("fC3dv", r3dv[:], c1[:, dD], T)
            add_dep_helper(i5.ins, pk_dma.ins, reason="p3 bitcast read")
            add_dep_helper(i6.ins, pk_dma.ins, reason="p3 bitcast read")
            scores("h3", fS3dv, fC3dv, c3e[:], s3e[:])

            # freq w2 (ACT pair lands after sins; folds direct)
            with nc.named_scope("fold_w2"):
                fSw2dv, i7 = tsm("fSw2dv", sw2[:, dD], scal(9), T)
                fCw2dv, i8 = tsm("fCw2dv", cw2[:, dD], scal(9), T)
            add_dep_helper(i7.ins, pk_dma.ins, reason="p3 bitcast read")
            add_dep_helper(i8.ins, pk_dma.ins, reason="p3 bitcast read")
            scores("w2", fSw2dv, fCw2dv, cw2[:, dE], sw2[:, dE])

            # freq 4w1: s4' = s2'c2, c4 = 1-8s2'^2
            with nc.named_scope("harm_h4"):
                s4pe, _ = tt("s4pe", s2pe[:], c2e[:], S)
                qqe, _ = tt("qqe", s2pe[:], s2pe[:], S)
                c4e, _ = ts2("c4e", qqe[:], -8.0, 1.0, S)
                c2dv, i9 = ts2("c2dv", qd[:], scal(6), scal(7), T)
                fS4dv, _ = tt("fS4dv", c2dv[:], s2pd[:], T)
                qqd, _ = tt("qqd", s2pd[:], s2pd[:], T)
                fC4dv, i10 = ts2("fC4dv", qqd[:], scal(8), scal(7), T)
            add_dep_helper(i9.ins, pk_dma.ins, reason="p3 bitcast read")
            add_dep_helper(i10.ins, pk_dma.ins, reason="p3 bitcast read")
            scores("h4", fS4dv, fC4dv, c4e[:], s4pe[:], last=True)

            # ---- softmax + decoder-mask scale + store; outputs split
            # across the two HWDGE rings ----
            ex, rs = {}, {}
            for tb in range(2):
                with nc.named_scope(f"exp_{tb}"):
                    ex[tb] = wrk.tile([128, S], BF16, name=f"ex{tb}")
                    rs[tb] = wrk.tile([128, 1], F32, name=f"rs{tb}")
                    nc.scalar.activation(
                        ex[tb][:], sc[tb][:], AF.Exp, accum_out=rs[tb][:])
            for tb in range(2):
                with nc.named_scope(f"scale_{tb}"):
                    ri = wrk.tile([128, 1], F32, name=f"ri{tb}")
                    nc.vector.reciprocal(ri[:], rs[tb][:])
                    fac = wrk.tile([128, 1], F32, name=f"fac{tb}")
                    fac_i = nc.vector.tensor_tensor(
                        fac[:], ri[:], scal(10 + tb), mybir.AluOpType.mult)
                    add_dep_helper(fac_i.ins, pk_dma.ins, reason="p3 bitcast read")
                    ot = wrk.tile([128, S], BF16, name=f"ot{tb}")
                    nc.vector.tensor_scalar_mul(ot[:], ex[tb][:], fac[:])
                    eng = nc.sync if tb == 0 else nc.scalar
                    eng.dma_start(out_d[tb * 128:(tb + 1) * 128, :], ot[:])

    nc.compile()
    _CACHE["nc"] = nc
    return nc


def kernel(encoder_output, decoder_output, W1, W2, v, enc_lens, dec_lens):
    global LAST_EXEC_NS
    from concourse.bass_utils import run_bass_kernel_spmd
    import ml_dtypes

    BF = ml_dtypes.bfloat16
    enc = np.asarray(encoder_output, dtype=np.float32)
    dec = np.asarray(decoder_output, dtype=np.float32)
    W1 = np.asarray(W1, dtype=np.float32)
    W2 = np.asarray(W2, dtype=np.float32)
    v = np.asarray(v, dtype=np.float32)
    enc_lens = np.asarray(enc_lens)
    dec_lens = np.asarray(dec_lens)

    v64 = v.astype(np.float64)
    vb = [(v64 * BEFF[k]).astype(np.float32) for k in range(5)]
    cols = [
        vb[0],                 # 0: vb0
        vb[1],                 # 1: vb1
        -2.0 * vb[1],          # 2: m2vb1
        -4.0 * vb[2],          # 3: m4vb2
        3.0 * vb[2],           # 4: p3vb2
        vb[2],                 # 5: vb2
        -2.0 * vb[3],          # 6: m2vb3
        vb[3],                 # 7: vb3
        -8.0 * vb[3],          # 8: m8vb3
        vb[4],                 # 9: vb4
    ]

    in_maps = []
    for b in range(B):
        p1 = np.concatenate([W2, dec[:, b, :].T], axis=1).astype(BF)
        p2 = np.ascontiguousarray(
            np.concatenate([W1, enc[:, b, :].T], axis=1).astype(BF))
        dm = (np.arange(T) < int(dec_lens[b])).astype(np.float32)
        dmr = dm.reshape(H, 2, order="F")
        p3 = np.stack(cols + [dmr[:, 0], dmr[:, 1]], axis=1).astype(np.float32)
        p3 = np.ascontiguousarray(p3)
        p3_bits = p3.view(np.uint16).view(BF)  # raw f32 bits as bf16 pairs
        pack1 = np.ascontiguousarray(np.concatenate([p1, p3_bits], axis=1))
        em = np.where(
            np.arange(S)[None, :] < int(enc_lens[b]), 0.0, NEG_BIG
        ).astype(BF)
        in_maps.append({"pack1": pack1, "pack2": p2, "encmask": em})

    trace = os.environ.get("KERNEL_TRACE", "0") == "1"
    if trace:
        _try_install_trace_hook()
    nc = _build()
    ncores = int(os.environ.get("KERNEL_CORES", str(B)))
    res = run_bass_kernel_spmd(nc, in_maps[:ncores], core_ids=list(range(ncores)), trace=trace)
    if trace:
        LAST_EXEC_NS = res.exec_time_ns
        _CACHE["last_res"] = res

    out = np.zeros((T, B, S), dtype=np.float32)
    for b in range(ncores):
        out[:, b, :] = np.asarray(res.results[b]["out"], dtype=np.float32)
    return out
